# revision 11
# baseline (speedup 1.0000x reference)
"""MoE adapter layer (top-2 of 8 LoRA experts) for Trainium2, 8 NeuronCores.

Strategy
--------
Data-parallel over B: core b handles batch b (B == 8 == n_cores).

The reference's gating softmaxes masked logits where non-top-k entries are
-inf, so their gates are *exactly* 0.0 and only the top-2 experts per batch
contribute to the output.  Routing (an [8,1024]x[1024,8] matmul + top-2 +
softmax) is done on the host as part of input sharding; the two selected
rank-64 LoRAs of a batch are stacked into a single rank-128 LoRA, with the
gate weights folded into the up-projection:

    out[b].T = x[b].T + BwS_b @ (AS_b @ x[b].T)

where AS_b = concat(A[e0], A[e1]) is [128, H] and
BwS_b = concat(g0*Bw[e0], g1*Bw[e1]) is [H, 128].

On-device (per core), everything is done transposed (x.T is [H, L]) so the
contraction dim H lands on SBUF partitions for matmul 1 and the rank-128 mid
result lands on partitions for matmul 2.

The kernel is HBM-bandwidth bound (~9 MB of bf16 traffic, ~358 GB/s/core
HBM cap) with a fixed ~10.5 us harness overhead (NEFF preamble + a ~7.5 us
all-semaphore-zeroing NEFF epilogue), so the schedule is built to keep the
HBM stream dense end-to-end:

* Per-ring DMA throughput is limited by a ~0.7 us inter-transfer gap, so x
  is loaded in 512 KiB "quad" tiles (4 h-chunks interleaved, [128,4,512])
  -> ~246 GB/s per HWDGE ring, two rings (sync+scalar) saturate HBM.
* Both weights ship as ONE 512 KiB transfer (wAB) first on the sync ring;
  lb0's x arrives as four 256 KiB pair tiles split across both rings so the
  first mm1 starts ~3 us after the stream opens and the first output quad
  is in the store queue ~6 us in -- the 4 MiB store stream then overlaps
  the remaining input stream instead of serializing after it.
* Every x/out tile has its own buffer slot (no pool recycling): input DMA
  issue never waits on compute.
* Stores stream per output quad [128,4,512] (512 KiB): early quads on the
  gpsimd SWDGE ring (HWDGE rings still carry input), late quads on
  sync/scalar once their input share has drained.
* mm1 contracts quad-arrival-ordered; mm2 folds the residual into the PE
  (identity matmul) for one pair per quad (ScalarE moves it) and adds it on
  VectorE during the PSUM->SBUF move for the other, so neither mover
  engine paces the stream.
* A chain of warm-up matmuls at kernel start keeps the PE busy through the
  load phase so HAM unthrottles it to 2.4 GHz before the real matmuls run.
"""

import os

import numpy as np

B, L, H = 8, 2048, 1024
E, TOPK, R = 8, 2, 64
P = 128
NF = 512  # max matmul moving free dim (PSUM bank = 512 f32/partition)
KH = H // P  # 8 contraction chunks over H
HC = H // P  # 8 output-row chunks over H
N_WARM = int(os.environ.get("MOE_N_WARM", "20"))  # PE warm-up matmuls
# h-chunk pairs j whose residual goes through the PE as an identity matmul
# with a ScalarE copy as the mover (offloads VectorE at the cost of PE time);
# the rest add the residual on VectorE during the PSUM->SBUF move.
ACT_J = (1, 3)

# L blocks: (col_start, width).  The first blocks are narrow so the first
# mm1/mm2/store chain launches as early into the input stream as possible
# (the store stream is what overlaps the remaining input DMA); the rest are
# full PSUM-bank width.
BLOCKS = ((0, 256), (256, 256), (512, 512), (1024, 512), (1536, 512))
NLB = len(BLOCKS)
# earliest-sim-start pins (ms) per block for mm1 / mm2: the real-time x
# arrival estimates at ~180 GB/s per ring while both rings stream.  These
# keep the tile scheduler's static per-engine order aligned with real DMA
# arrival (its own greedy idle-filling would hoist mm1(lb+1), whose inputs
# arrive late, ahead of mm2(lb), stalling the in-order PE stream at runtime).
MM1_PIN = (0.0050, 0.0064, 0.0092, 0.0121, 0.0150)
MM2_PIN = (0.0055, 0.0069, 0.0097, 0.0126, 0.0155)

# store-quad (lb, qq) -> issuing engine.  gpsimd (SWDGE ring Q0) carries the
# early stores while sync/scalar HWDGE rings still carry input; the late
# quads go to sync/scalar, whose input share has drained by the time these
# issue.
STORE_ENG = {
    (0, 0): "gpsimd",
    (0, 1): "gpsimd",
    (1, 0): "gpsimd",
    (1, 1): "gpsimd",
    (2, 0): "gpsimd",
    (2, 1): "scalar",
    (3, 0): "sync",
    (3, 1): "scalar",
    (4, 0): "sync",
    (4, 1): "gpsimd",
}

# dtype config: "bf16" (bf16 I/O+matmuls, f32 PSUM accumulate),
# "f32r" (f32 I/O, float32r matmuls), "f32" (exact f32 matmuls, 4x slower PE)
CFG = os.environ.get("MOE_KERNEL_CFG", "bf16")

_BUILD_CACHE: dict = {}


def _dtypes(cfg):
    import concourse.mybir as mybir

    f32 = mybir.dt.float32
    if cfg == "bf16":
        bf16 = mybir.dt.bfloat16
        return dict(io=bf16, mm=bf16, mid=bf16, out=bf16, np_io=np.dtype("bfloat16"))
    if cfg == "f32r":
        f32r = mybir.dt.float32r
        return dict(io=f32r, mm=f32r, mid=f32r, out=f32, np_io=np.dtype(np.float32))
    if cfg == "f32":
        return dict(io=f32, mm=f32, mid=f32, out=f32, np_io=np.dtype(np.float32))
    raise ValueError(cfg)


def _build(cfg):
    """Build the single-core Bass program (same program SPMD on all 8 cores)."""
    if cfg in _BUILD_CACHE:
        return _BUILD_CACHE[cfg]

    import concourse.bacc as bacc
    import concourse.mybir as mybir
    from concourse.masks import make_identity
    from concourse.tile import TileContext

    dts = _dtypes(cfg)
    f32 = mybir.dt.float32

    # Bacc (not raw Bass): its compile() runs generate_event_semaphores,
    # which legalizes to TRN2's one-sync-wait-per-instruction limit.
    nc = bacc.Bacc()
    xT = nc.dram_tensor("xT", [H, L], dts["io"], kind="ExternalInput")
    # wAB: [p, 0:1024]  = AS.T pre-tiled on host as [p, k*128+m] = AS.T[k*128+p, m]
    #      [p, 1024:2048] = BwS.T (gates folded in)
    wAB = nc.dram_tensor("wAB", [P, 2 * H], dts["mm"], kind="ExternalInput")
    yT = nc.dram_tensor("yT", [H, L], dts["out"], kind="ExternalOutput")

    def as_f32(ap):
        return ap.bitcast(f32) if ap.dtype == mybir.dt.float32r else ap

    with TileContext(nc) as tc:
        with (
            tc.tile_pool(name="wpool", bufs=1) as wpool,
            tc.tile_pool(name="xpool", bufs=1) as xpool,
            tc.tile_pool(name="midpool", bufs=3) as midpool,
            tc.tile_pool(name="outpool", bufs=1) as outpool,
            tc.tile_pool(name="psA", bufs=2, space="PSUM") as psA,
            tc.tile_pool(name="psB", bufs=3, space="PSUM") as psB,
        ):
            # Weights at the head of each ring (256 KiB each): wA leads the
            # sync ring (mm1 needs it first), wB leads the scalar ring (mm2
            # needs it ~1 us later).  Both rings then carry lb0's pair tiles
            # so the first L-block completes as early as possible.
            wAt = wpool.tile([P, H], dts["mm"], name="wAt")
            nc.sync.dma_start(out=wAt, in_=wAB[:, 0:H])
            wBt = wpool.tile([P, H], dts["mm"], name="wBt")
            nc.scalar.dma_start(out=wBt, in_=wAB[:, H : 2 * H])

            def wA_sl(k):
                return wAt[:, k * P : (k + 1) * P]

            def wB_sl(h):
                return wBt[:, h * P : (h + 1) * P]

            # x tiles: per L-block, two quad tiles [128, 4, w] (4 h-chunks
            # interleaved), quad 0 on the sync ring, quad 1 on scalar, in
            # block order.  Every tile has its own slot (tag), so loads
            # never wait on compute.
            xseg = {}  # (k, lb) -> [128, w] AP view of x row-chunk k
            xq = {}  # (q, lb) -> quad tile [128, 4, w]

            for lb, (c0, w) in enumerate(BLOCKS):
                for q in range(2):  # quads: chunks 4q..4q+3
                    t = xpool.tile(
                        [P, 4, w],
                        dts["io"],
                        tag=f"xq{q}l{lb}",
                        name=f"xq{q}l{lb}",
                        bufs=1,
                    )
                    eng = nc.sync if q == 0 else nc.scalar
                    eng.dma_start(
                        out=t,
                        in_=xT[4 * q * P : (4 * q + 4) * P, c0 : c0 + w].rearrange(
                            "(four p) c -> p four c", four=4
                        ),
                    )
                    xq[q, lb] = t
                    for f in range(4):
                        xseg[4 * q + f, lb] = t[:, f, :]

            def xpair(j, lb):
                """[128, 2, w] view of x row-chunk pair j (chunks 2j, 2j+1)."""
                q, f = divmod(2 * j, 4)
                return xq[q, lb][:, f : f + 2, :]

            # identity: warm-up operand + PE-side residual accumulate weights
            ident = wpool.tile([P, P], dts["mm"], name="ident")
            make_identity(nc, ident)

            # PE warm-up: a dependency-free chain of small matmuls that runs
            # while the x DMAs stream in, flipping HAM to 8/8 (2.4 GHz).
            warm = wpool.tile([P, P], dts["mm"], name="warm")
            nc.vector.memset(warm, 1.0)
            warm_ps = psA.tile([P, NF], f32, tag="mid_ps", name="warm_ps")
            for _ in range(N_WARM):
                nc.tensor.matmul(
                    warm_ps[:, :P], lhsT=warm, rhs=warm, start=True, stop=True
                )

            mid_sbs = {}

            def do_mm1(lb):
                # mm1: mid[128, w] = AS @ xT[:, lb block], contract over H
                w = BLOCKS[lb][1]
                mid_ps = psA.tile([P, NF], f32, name="mid_ps")
                for k in range(KH):
                    nc.tensor.matmul(
                        mid_ps[:, :w],
                        lhsT=wA_sl(k),
                        rhs=xseg[k, lb],
                        start=(k == 0),
                        stop=(k == KH - 1),
                    )
                mid_sb = midpool.tile([P, NF], dts["mid"], name="mid_sb")
                # alternate the mid mover so neither DVE nor ACT paces the
                # mm2 phase alone
                if lb % 2 == 0:
                    nc.scalar.copy(out=as_f32(mid_sb[:, :w]), in_=mid_ps[:, :w])
                else:
                    nc.vector.tensor_copy(out=as_f32(mid_sb[:, :w]), in_=mid_ps[:, :w])
                mid_sbs[lb] = mid_sb[:, :w]

            def do_mm2(lb):
                # mm2 + residual add + store, streamed per h-quad so output
                # DMA overlaps the remaining input DMA.  Movers operate on
                # h-chunk pairs [128, 2*NF]; stores ship h-quads [128, 4*NF].
                c0, w = BLOCKS[lb]
                ls = slice(c0, c0 + w)
                mid_sb = mid_sbs[lb]
                for qq in range(2):  # output quad: h-chunks 4qq..4qq+3
                    out_q = outpool.tile(
                        [P, 4, w],
                        dts["out"],
                        tag=f"o{qq}l{lb}",
                        name=f"o{qq}l{lb}",
                        bufs=1,
                    )
                    for jj in range(2):  # pair within quad
                        j = 2 * qq + jj
                        out_ps = psB.tile(
                            [P, 2, w], f32, name="out_ps", padded_shape=[P, 2, NF]
                        )
                        on_act = j in ACT_J
                        for i in range(2):
                            h = 2 * j + i
                            nc.tensor.matmul(
                                out_ps[:, i, :],
                                lhsT=wB_sl(h),
                                rhs=mid_sb,
                                start=True,
                                stop=not on_act,
                            )
                            if on_act:
                                # residual folded into PE; ScalarE moves it
                                nc.tensor.matmul(
                                    out_ps[:, i, :],
                                    lhsT=ident,
                                    rhs=xseg[h, lb],
                                    start=False,
                                    stop=True,
                                )
                        dst = out_q[:, 2 * jj : 2 * jj + 2, :]
                        if on_act:
                            nc.scalar.copy(out=dst, in_=out_ps)
                        else:
                            # residual added during the PSUM->SBUF move
                            nc.vector.tensor_add(
                                out=dst, in0=out_ps, in1=as_f32(xpair(j, lb))
                            )
                    eng = getattr(nc, STORE_ENG[(lb, qq)])
                    eng.dma_start(
                        out=yT[4 * qq * P : (4 * qq + 4) * P, ls].rearrange(
                            "(four p) c -> p four c", four=4
                        ),
                        in_=out_q,
                    )

            # in-order emission: mm1(lb) then mm2(lb) keeps the store stream
            # as early as possible (it's what overlaps with input DMA).  The
            # tile scheduler's greedy idle-filling would otherwise hoist
            # mm1(lb+1) matmuls (whose x quads arrive LATE on the rings)
            # ahead of mm2(lb) (whose inputs are already on-chip), stalling
            # the in-order PE stream at runtime and delaying the store
            # overlap; pin each phase's earliest sim start to its real
            # input-arrival estimate (~180 GB/s per ring while both stream).
            for lb in range(NLB):
                tc.tile_set_cur_wait(MM1_PIN[lb])
                do_mm1(lb)
                tc.tile_set_cur_wait(MM2_PIN[lb])
                do_mm2(lb)

    nc.compile()
    _BUILD_CACHE[cfg] = nc
    return nc


def _route(x, Wr):
    """Host-side gating, mirroring the reference's noisy-top-k (eval) math."""
    cls = x[:, 0, :].astype(np.float32)  # [B, H]
    logits = cls @ Wr.T.astype(np.float32)  # [B, E]
    idx = np.argsort(-logits, axis=1, kind="stable")[:, :TOPK]  # [B, K] desc
    vals = np.take_along_axis(logits, idx, axis=1)
    e = np.exp(vals - vals.max(axis=1, keepdims=True))
    gates = e / e.sum(axis=1, keepdims=True)  # [B, K]
    return idx, gates.astype(np.float32)


def _ensure_ntff_hook_importable():
    """run_bass_kernel_spmd(trace=True) does a bare import of
    antenv.axon_hooks; some images lack it. Pre-install a shim (backed by the
    blessed ctypes NTFF hook when available) so tracing degrades gracefully
    instead of raising."""
    import sys

    try:
        from antenv.axon_hooks import get_axon_ntff_profile_hook  # noqa: F401

        return
    except ImportError:
        pass
    import types

    hook = None
    try:
        from trn_agent_boot.trn_boot import _ntff_profile_via_ctypes

        hook = _ntff_profile_via_ctypes("/opt/axon/libaxon_pjrt.so")
    except Exception:
        hook = None
    mod = types.ModuleType("antenv.axon_hooks")
    mod.get_axon_ntff_profile_hook = lambda: hook
    mod.set_axon_ntff_profile_hook = lambda h: None
    sys.modules["antenv.axon_hooks"] = mod


def kernel(x, Wr, A, Bw, _trace=False, _cfg=None):
    from concourse.bass_utils import run_bass_kernel_spmd

    _ensure_ntff_hook_importable()

    cfg = _cfg or CFG
    dts = _dtypes(cfg)
    np_io = dts["np_io"]

    x = np.asarray(x, dtype=np.float32)
    Wr = np.asarray(Wr, dtype=np.float32)
    A = np.asarray(A, dtype=np.float32)
    Bw = np.asarray(Bw, dtype=np.float32)

    idx, gates = _route(x, Wr)

    in_maps = []
    for b in range(B):
        e0, e1 = int(idx[b, 0]), int(idx[b, 1])
        g0, g1 = np.float32(gates[b, 0]), np.float32(gates[b, 1])
        AS = np.concatenate([A[e0], A[e1]], axis=0)  # [128, H]
        BwS = np.concatenate([g0 * Bw[e0], g1 * Bw[e1]], axis=1)  # [H, 128]
        # wA pre-tiled: [p, k*128+m] = AS.T[k*128+p, m] = AS[m, k*128+p]
        wAp = AS.T.reshape(KH, P, P).transpose(1, 0, 2).reshape(P, KH * P)
        wABp = np.ascontiguousarray(np.concatenate([wAp, BwS.T], axis=1))
        in_maps.append(
            {
                "xT": np.ascontiguousarray(x[b].T).astype(np_io),
                "wAB": wABp.astype(np_io),
            }
        )

    nc = _build(cfg)
    res = run_bass_kernel_spmd(
        nc,
        in_maps,
        core_ids=list(range(B)),
        trace=_trace,
        **({"trace_cores": list(range(B))} if _trace else {}),
    )

    out = np.empty((B, L, H), dtype=np.float32)
    for b in range(B):
        out[b] = res.results[b]["yT"].astype(np.float32).T
    if _trace:
        kernel._last_result = res
    return out
